# revision 1
# baseline (speedup 1.0000x reference)
"""Trainium2 Bass kernel for nn_MemoryReader.

Reference computation (per batch b):
    mi = mk.reshape(CK, N);  qi = qk.reshape(CK, P) / sqrt(CK)
    S  = mi.T @ qi                      # [N, P] affinity logits
    A  = softmax(S, axis=0)             # over memory axis N
    mem = mv.reshape(CV, N) @ A         # [CV, P]
    out = concat([mem, qv], axis=channel)

Sharding: 8 cores = (4 batches) x (2 halves of the memory axis N).
Each core computes, for its (b, half):
    E      = exp(S_half)                        # no max subtraction (logits ~ N(0,1))
    memT   = E.T @ mv_half.T                    # [P, CV] unnormalized numerator
    lsum   = ones @ E                           # [1, P] denominator part
The host combines: mem = (mem_un_0 + mem_un_1) / (lsum_0 + lsum_1), then
concats qv (pure passthrough). No on-device collectives needed.

Device layout notes:
  - E is produced directly in [n(partition), p(free)] layout by computing
    S = mk_tile.T @ qk (lhsT = mk slice, K=CK=64 on partitions).
  - The second matmul contracts over n, so both operands need n on
    partitions: mv is pre-transposed ON THE HOST into [128, NT, CV]
    (partition-major tiles), making the device program transpose-free.
  - 1/sqrt(CK) is folded into the exp activation's free affine scale.
"""

import numpy as np
import ml_dtypes

import concourse.tile as tile
from concourse import bacc, mybir
from concourse.bass_utils import run_bass_kernel_spmd

# Problem shape (hardcoded per contract)
B, CK, CV, T, H, W = 4, 64, 512, 8, 30, 54
N = T * H * W          # 12960 memory positions
P = H * W              # 1620 query positions
NHALF = N // 2         # 6480 per core
NT = (NHALF + 127) // 128   # 51 n-tiles (last has 80 rows)
NLAST = NHALF - (NT - 1) * 128  # 80
NPAD = NT * 128        # 6528
# p-axis chunking: chunks of <=512 (one PSUM bank for mm1 out), each chunk
# sliced into 128-wide pieces that serve as mm2 stationary weights. The
# small 84-wide chunk runs LAST: its ACT-bound low-PE-duty cadence overlaps
# the output-DMA tail. (ps, width, n_slices); global slice id = ps//128 + sl.
PCHUNKS = [(0, 512, 4), (512, 512, 4), (1024, 512, 4), (1536, 84, 1)]
NSL = 13

# Matmul precision mode: "bf16" (1 cyc/col), "f32r" (fp32 data, ~1 cyc/col
# at free>=256), "f32" (4 cyc/col).
MM_MODE = "bf16"

_CACHE = {}


def _mm_dtype():
    return {
        "bf16": mybir.dt.bfloat16,
        "f32r": mybir.dt.float32r,
        "f32": mybir.dt.float32,
    }[MM_MODE]


def _np_dtype():
    return ml_dtypes.bfloat16 if MM_MODE == "bf16" else np.float32


def _mm_ap(ap):
    """Operand view handed to the tensor engine."""
    return ap


def _f32view(ap):
    """float32 view for vector-engine reads (f32r is fp32 bits)."""
    if MM_MODE == "f32r":
        return ap.bitcast(mybir.dt.float32)
    return ap


def _build_program():
    dt = _mm_dtype()
    f32 = mybir.dt.float32
    # Bacc (not plain Bass): its compile() runs generate_event_semaphores,
    # which splits multi-wait sync_info onto EventSemaphore instructions
    # (TRN2 allows only one wait per regular instruction).
    nc = bacc.Bacc(None, target_bir_lowering=False, debug=False)

    # NOTE on mm1 structure: the contraction dim is CK=64, but mk/qk are
    # zero-padded to K=128 on the host. Matmul time is column-bound (K is
    # free), and only full-row (K=128) LDWEIGHTS go through the background
    # weight buffer — K=64 weight loads (and tile_position row-packed pairs,
    # which were tried) serialize ~200ns per matmul on the weight port.
    mk_d = nc.declare_dram_parameter("mk", [128, NT, 128], dt, isOutput=False)
    qk_d = nc.declare_dram_parameter("qk", [128, P], dt, isOutput=False)
    mvt_d = nc.declare_dram_parameter("mvT", [128, NT, CV], dt, isOutput=False)
    # outputs in transposed layout: memT[p, v]; lsum packed [row, slice] with
    # l[p] at row=p%128, slice=p//128
    mem_d = nc.declare_dram_parameter("memT", [P, CV], f32, isOutput=True)
    l_d = nc.declare_dram_parameter("lsum", [128, 2 * NSL], f32, isOutput=True)

    with tile.TileContext(nc) as tc:
        with (
            tc.tile_pool(name="singles", bufs=1) as singles,
            tc.tile_pool(name="epool", bufs=4) as epool,
            tc.tile_pool(name="opool", bufs=8) as opool,
            tc.tile_pool(name="olpool", bufs=2) as olpool,
            tc.tile_pool(name="rpool", bufs=2) as rpool,
            tc.tile_pool(name="spsum", bufs=3, space="PSUM") as spsum,
            tc.tile_pool(name="accpsum", bufs=4, space="PSUM") as accpsum,
            tc.tile_pool(name="lpsum", bufs=1, space="PSUM") as lpsum,
        ):
            # fp32 ones for the (tiny, fp32) cross-partition R sum matmuls
            ones32 = singles.tile([128, 2], f32, name="ones32")
            nc.vector.memset(ones32, 1.0)
            qk_sb = singles.tile([128, P], dt)
            mk_sb = singles.tile([128, NT, 128], dt)
            mvt_sb = singles.tile([128, NT, CV], dt)
            # interleave the loads in consumption order: qk slivers per chunk
            # (the first chunk's is tiny, so compute starts immediately), then
            # mk tiles, then mvT. Each weight-tile read depends on exactly one
            # DMA (avoids multi-sem wait explosion).
            for ps_, w_, _ in PCHUNKS:
                nc.sync.dma_start(
                    out=qk_sb[:, ps_:ps_ + w_], in_=qk_d[:, ps_:ps_ + w_]
                )
            nc.sync.dma_start(out=mk_sb[:, 0:13, :], in_=mk_d[:, 0:13, :])
            NTG = 3
            for g in range(0, 6, NTG):
                nc.sync.dma_start(
                    out=mvt_sb[:, g:g + NTG, :], in_=mvt_d[:, g:g + NTG, :]
                )
            for g in range(13, NT, 13):
                g1 = min(g + 13, NT)
                nc.sync.dma_start(out=mk_sb[:, g:g1, :], in_=mk_d[:, g:g1, :])
            for g in range(6, NT, NTG):
                g1 = min(g + NTG, NT)
                nc.sync.dma_start(
                    out=mvt_sb[:, g:g1, :],
                    in_=mvt_d[:, g:g1, :],
                )

            # Warm-up: full-size (M=K=128) matmuls on a memset tile, depending
            # on no DMA. They run while the input DMAs land, filling the
            # initial PE idle gap AND releasing the HAM clock throttle (~3.4us
            # of sustained activity; tiny-M matmuls don't count as PE-busy).
            warmw = singles.tile([128, 128], mybir.dt.bfloat16, name="warmw")
            nc.vector.memset(warmw, 1.0)
            warm = lpsum.tile([128, 128], f32, tag="lacc", name="warm")
            for _ in range(48):
                nc.tensor.matmul(
                    warm,
                    lhsT=warmw,
                    rhs=warmw,
                    start=True,
                    stop=True,
                )

            def issue_mm1(ps, w, nt, s_pool_tiles):
                nsz = 128 if nt < NT - 1 else NLAST
                s = spsum.tile([128, 512], f32, tag="s", name="s")
                nc.tensor.matmul(
                    s[:nsz, :w],
                    lhsT=_mm_ap(mk_sb[:, nt, :nsz]),
                    rhs=_mm_ap(qk_sb[:, ps:ps + w]),
                    start=True,
                    stop=True,
                )
                s_pool_tiles[nt] = s

            for ci, (ps, w, nsl) in enumerate(PCHUNKS):
                sl0 = ps // 128  # global slice index of chunk's first slice
                acc = []
                for sl in range(nsl):
                    acc.append(accpsum.tile([128, CV], f32, tag="acc", name="acc"))
                # Denominator: R = sum over n-tiles of E, accumulated
                # elementwise on the (otherwise idle) vector engine; the
                # cross-partition sum happens in ONE matmul per slice at chunk
                # end. This keeps the PE inner loop free of the extra
                # weight-load per slice (the l matmuls' LDWEIGHTS were a
                # weight-port bottleneck).
                r_sb = rpool.tile([128, 512], f32, tag="r", name="r")
                nc.vector.memset(r_sb[:, :w], 0.0)
                # one psum bank for all slices' [pw, 2] column pairs (N=2:
                # N=1 is invalid for f32r; M=1 matmuls don't count as PE-busy
                # for the HAM clock gate). Only the first matmul of the bank
                # uses start=True (whole-bank has_written clear); later
                # slices rely on per-element overwrite-when-bit-unset.
                lacc = lpsum.tile([128, 2 * NSL], f32, tag="lacc", name="lacc")

                s_tiles = {}
                issue_mm1(ps, w, 0, s_tiles)
                for nt in range(NT):
                    if nt + 1 < NT:
                        issue_mm1(ps, w, nt + 1, s_tiles)
                    nsz = 128 if nt < NT - 1 else NLAST
                    s_cur = s_tiles.pop(nt)
                    e_sb = epool.tile([128, 512], dt, tag="e", name="e")
                    nc.scalar.activation(
                        out=e_sb[:nsz, :w],
                        in_=s_cur[:nsz, :w],
                        func=mybir.ActivationFunctionType.Exp,
                        scale=0.125,  # 1/sqrt(CK)
                    )
                    nc.vector.tensor_add(
                        out=r_sb[:nsz, :w],
                        in0=r_sb[:nsz, :w],
                        in1=_f32view(e_sb[:nsz, :w]),
                    )
                    first, last = nt == 0, nt == NT - 1
                    for sl in range(nsl):
                        pw = min(128, w - sl * 128)
                        el = e_sb[:nsz, sl * 128:sl * 128 + pw]
                        nc.tensor.matmul(
                            acc[sl][:pw],
                            lhsT=_mm_ap(el),
                            rhs=_mm_ap(mvt_sb[:nsz, nt, :]),
                            start=first,
                            stop=last,
                        )

                # cross-partition sum of R -> l, one matmul per slice
                for sl in range(nsl):
                    pw = min(128, w - sl * 128)
                    gsl = sl0 + sl
                    nc.tensor.matmul(
                        lacc[:pw, 2 * gsl:2 * gsl + 2],
                        lhsT=r_sb[:, sl * 128:sl * 128 + pw],
                        rhs=ones32,
                        start=sl == 0,
                        stop=sl == nsl - 1,
                        skip_group_check=True,
                    )

                for sl in range(nsl):
                    pw = min(128, w - sl * 128)
                    o_sb = opool.tile([128, CV], f32, tag="o", name="o")
                    nc.vector.tensor_copy(out=o_sb[:pw], in_=acc[sl][:pw])
                    p0 = ps + sl * 128
                    nc.sync.dma_start(out=mem_d[p0:p0 + pw, :], in_=o_sb[:pw])
                ol_sb = olpool.tile([128, 2 * NSL], f32, tag="ol", name="ol")
                nc.vector.tensor_copy(
                    out=ol_sb[:, 2 * sl0:2 * (sl0 + nsl)],
                    in_=lacc[:, 2 * sl0:2 * (sl0 + nsl)],
                )
                nc.sync.dma_start(
                    out=l_d[:, 2 * sl0:2 * (sl0 + nsl)],
                    in_=ol_sb[:, 2 * sl0:2 * (sl0 + nsl)],
                )

    _strip_same_engine_waits(nc)
    nc.compile()
    return nc


def _strip_same_engine_waits(nc):
    """Drop redundant same-engine semaphore waits on ACT/PE compute
    instructions.

    Each engine executes its queue in order, so an ACTIVATE waiting on the
    Activation engine's own completion semaphore (a WAW slot-reuse guard Tile
    emits conservatively) is a no-op — but TRN2 instructions hold only ONE
    wait, so the extra wait forces generate_event_semaphores to insert a
    separate EVENT_SEMAPHORE instruction that serializes the engine queue
    (~0.6us each on the scalar engine). DVE is left alone: its chains include
    genuine same-engine RAW dependencies.
    """
    prefixes = {
        "EngineType.Activation": "Activation_",
        "EngineType.PE": "PE_",
    }
    kinds = (mybir.InstActivation, mybir.InstMatmult, mybir.InstLdweights)
    for fn in nc.m.functions:
        for blk in fn.blocks:
            for inst in blk.instructions:
                si = getattr(inst, "sync_info", None)
                if si is None or not si.on_wait or not isinstance(inst, kinds):
                    continue
                pref = prefixes.get(str(getattr(inst, "engine", None)))
                if pref is None:
                    continue
                kept = [w for w in si.on_wait
                        if not str(getattr(w, "ant_name", "")).startswith(pref)]
                if len(kept) != len(si.on_wait):
                    si.on_wait = kept


def _get_program():
    if "nc" not in _CACHE:
        _CACHE["nc"] = _build_program()
    return _CACHE["nc"]


def _make_in_maps(mk, mv, qk):
    npdt = _np_dtype()
    mkf = np.ascontiguousarray(mk.reshape(B, CK, N))
    mvf = np.ascontiguousarray(mv.reshape(B, CV, N))
    qkf = np.ascontiguousarray(qk.reshape(B, CK, P))
    in_maps = []
    for core in range(8):
        b, half = core // 2, core % 2
        n0, n1 = half * NHALF, (half + 1) * NHALF
        mk_c = mkf[b, :, n0:n1].astype(npdt)          # [64, 6480]
        # zero-pad the contraction dim to 128 (see mm1 note in _build_program)
        mk_t = np.zeros((128, NT, 128), dtype=npdt)
        mk_t[:CK].reshape(CK, NT * 128)[:, :NHALF] = mk_c
        qk_c = np.zeros((128, P), dtype=npdt)
        qk_c[:CK] = qkf[b].astype(npdt)
        mvt = np.zeros((NPAD, CV), dtype=npdt)
        mvt[:NHALF] = mvf[b, :, n0:n1].T
        # partition-major tiles: [128, NT, CV], elem (p, t, v) = mvT[t*128+p, v]
        mvt_c = np.ascontiguousarray(mvt.reshape(NT, 128, CV).transpose(1, 0, 2))
        in_maps.append({"mk": np.ascontiguousarray(mk_t),
                        "qk": np.ascontiguousarray(qk_c),
                        "mvT": mvt_c})
    return in_maps


def _run(mk, mv, qk, qv, trace=False, **spmd_kwargs):
    nc = _get_program()
    in_maps = _make_in_maps(mk, mv, qk)
    res = run_bass_kernel_spmd(nc, in_maps, list(range(8)), trace=trace, **spmd_kwargs)
    out = np.empty((B, 2 * CV, P), dtype=np.float32)
    for b in range(B):
        m0, l0 = res.results[2 * b]["memT"], res.results[2 * b]["lsum"]
        m1, l1 = res.results[2 * b + 1]["memT"], res.results[2 * b + 1]["lsum"]
        # memT is [P, CV]; lsum [128, 2*NSL]: l[p] at [p % 128, 2*(p // 128)]
        lv = (l0 + l1)[:, 0::2].T.reshape(-1)[:P]
        out[b, :CV] = ((m0 + m1) / lv[:, None]).T
        out[b, CV:] = qv[b].reshape(CV, P)
    return out.reshape(B, 2 * CV, H, W), res


def kernel(mk, mv, qk, qv):
    out, _ = _run(np.asarray(mk), np.asarray(mv), np.asarray(qk), np.asarray(qv))
    return out



# revision 5
# speedup vs baseline: 1.4846x; 1.4846x over previous
"""Trainium2 Bass kernel for nn_MemoryReader.

Reference computation (per batch b):
    mi = mk.reshape(CK, N);  qi = qk.reshape(CK, P) / sqrt(CK)
    S  = mi.T @ qi                      # [N, P] affinity logits
    A  = softmax(S, axis=0)             # over memory axis N
    mem = mv.reshape(CV, N) @ A         # [CV, P]
    out = concat([mem, qv], axis=channel)

Sharding: 8 cores = (4 batches) x (2 halves of the memory axis N).
Each core computes, for its (b, half):
    E      = exp(0.125*S - 2)                   # fp8e4; the -2 bias keeps
                                                # E<=54 (TRN fp8e4 max 240)
                                                # and cancels in the softmax
    memT   = E.T @ [mvT | 1]                    # [P, 513]: col 512 = sum(E)
The host combines: mem = (num_0 + num_1) / (den_0 + den_1), then concats
qv (pure passthrough). No on-device collectives needed.

Key speed features vs the bf16 version:
  - mm2 runs in fp8e4 DoubleRow mode: each matmul contracts TWO 128-row
    n-tiles (lhsT/rhs get [K, 2, M] APs), ~2x column throughput.
  - The softmax denominator is a 513th "ones" column of mvT, accumulated
    by the same mm2 matmuls (split 256+257 to satisfy the one-PSUM-bank
    rule, sharing one LDWEIGHTS via a dedupe pass) - no vector-engine
    accumulation at all.
  - exp() is fused 4 n-tiles per ACT instruction (PSUM "squad" tiles) to
    amortize the ~185ns per-instruction access latency.
"""

import numpy as np
import ml_dtypes

import concourse.tile as tile
from concourse import bacc, mybir
from concourse.bass_utils import run_bass_kernel_spmd

# Problem shape (hardcoded per contract)
B, CK, CV, T, H, W = 4, 64, 512, 8, 30, 54
N = T * H * W          # 12960 memory positions
P = H * W              # 1620 query positions
NHALF = N // 2         # 6480 per core
NT = (NHALF + 127) // 128   # 51 n-tiles (last has 80 rows)
NLAST = NHALF - (NT - 1) * 128  # 80
NTP = NT + 1           # pad to even tile count for DoubleRow pairing
MVW = 528              # mvT free width: 512 mv + 1 ones + 15 pad (16B align)
CVA = 513              # real mm2 output width (512 mv + 1 denominator)
ASPL = 256             # a-half columns (b-half = 257); each fits a PSUM bank
EXP_BIAS = -2.0        # exp(0.125*s - 2): range safety for fp8e4

# p-axis chunks of 256 (2 slices of 128 each; last chunk 84).
PCH = [(0, 256), (256, 256), (512, 256), (768, 256), (1024, 256), (1280, 256),
       (1536, 84)]
QUADS = [(0, 4), (4, 4), (8, 4), (12, 4), (16, 4), (20, 4), (24, 4), (28, 4),
         (32, 4), (36, 4), (40, 4), (44, 4), (48, 3)]

DEDUPE_LDW = True

_CACHE = {}


def _build_program():
    f8 = mybir.dt.float8e4
    bf16 = mybir.dt.bfloat16
    f32 = mybir.dt.float32
    DR = mybir.MatmulPerfMode.DoubleRow
    nc = bacc.Bacc(None, target_bir_lowering=False, debug=False)

    # mk/qk zero-padded to K=128 on the host: full-row LDWEIGHTS go through
    # the background weight buffer (K=64 loads serialize on the weight port).
    mk_d = nc.declare_dram_parameter("mk", [128, NT, 128], bf16, isOutput=False)
    qk_d = nc.declare_dram_parameter("qk", [128, P], bf16, isOutput=False)
    mvt_d = nc.declare_dram_parameter("mvT", [128, NTP, MVW], f8, isOutput=False)
    mem_d = nc.declare_dram_parameter("memT", [P, CVA], f32, isOutput=True)

    with tile.TileContext(nc) as tc:
        with (
            tc.tile_pool(name="singles", bufs=1) as singles,
            tc.tile_pool(name="epool", bufs=3) as epool,
            tc.tile_pool(name="opool", bufs=4) as opool,
            tc.tile_pool(name="spsum", bufs=2, space="PSUM") as spsum,
            tc.tile_pool(name="apsum", bufs=2, space="PSUM") as apsum,
            tc.tile_pool(name="bpsum", bufs=2, space="PSUM") as bpsum,
        ):
            qk_sb = singles.tile([128, P], bf16)
            mk_sb = singles.tile([128, NT, 128], bf16)
            mvt_sb = singles.tile([128, NTP, MVW], f8)
            # Loads in consumption order; each weight-tile read depends on
            # exactly one DMA (avoids multi-sem wait explosion).
            for ps_, w_ in PCH:
                nc.sync.dma_start(
                    out=qk_sb[:, ps_:ps_ + w_], in_=qk_d[:, ps_:ps_ + w_]
                )
            nc.sync.dma_start(out=mk_sb[:, 0:13, :], in_=mk_d[:, 0:13, :])
            for g in range(0, 6, 3):
                nc.sync.dma_start(
                    out=mvt_sb[:, g:g + 3, :], in_=mvt_d[:, g:g + 3, :]
                )
            for g in range(13, NT, 13):
                g1 = min(g + 13, NT)
                nc.sync.dma_start(out=mk_sb[:, g:g1, :], in_=mk_d[:, g:g1, :])
            for g in range(6, NTP, 3):
                g1 = min(g + 3, NTP)
                nc.sync.dma_start(
                    out=mvt_sb[:, g:g1, :], in_=mvt_d[:, g:g1, :]
                )

            # Warm-up: full-size matmuls on a memset tile, depending on no
            # DMA. They fill the initial PE idle gap AND release the HAM
            # clock throttle (~3.4us of sustained activity needed).
            warmw = singles.tile([128, 128], bf16, name="warmw")
            nc.vector.memset(warmw, 1.0)
            bias_sb = singles.tile([128, 1], f32, name="bias")
            nc.vector.memset(bias_sb, EXP_BIAS)
            warm = spsum.tile([128, 128], f32, tag="s", name="warm")
            for _ in range(48):
                nc.tensor.matmul(warm, lhsT=warmw, rhs=warmw,
                                 start=True, stop=True)

            # Flat software pipeline over (chunk, quad) units: issue unit
            # u+1's mm1 before unit u's mm2 so the PE queue never stalls on
            # the ACT->mm2 dependency.
            units = [(ci, qi) for ci in range(len(PCH)) for qi in range(len(QUADS))]
            squads = {}
            e4s = {}
            accs = {}

            def issue_mm1(u):
                ci, qi = units[u]
                ps, w = PCH[ci]
                q0, qn = QUADS[qi]
                s = spsum.tile([128, 4, ASPL], f32, tag="s", name="s")
                for j in range(qn):
                    nt = q0 + j
                    nsz = 128 if nt < NT - 1 else NLAST
                    nc.tensor.matmul(
                        s[:nsz, j, :w],
                        lhsT=mk_sb[:, nt, :nsz],
                        rhs=qk_sb[:, ps:ps + w],
                        start=True,
                        stop=True,
                    )
                squads[u] = s

            def issue_act(u):
                ci, qi = units[u]
                ps, w = PCH[ci]
                q0, qn = QUADS[qi]
                s = squads.pop(u)
                e4 = epool.tile([128, 4, ASPL], f8, tag="e", name="e")
                # rows 80:128 of the tri-quad's last tile hold stale PSUM;
                # exp of garbage lands in e4 rows the matmuls never read.
                nc.scalar.activation(
                    out=e4[:, 0:qn, :w],
                    in_=s[:, 0:qn, :w],
                    func=mybir.ActivationFunctionType.Exp,
                    scale=0.125,  # 1/sqrt(CK)
                    bias=bias_sb[:, :],
                )
                e4s[u] = e4

            def issue_mm2(u):
                ci, qi = units[u]
                ps, w = PCH[ci]
                q0, qn = QUADS[qi]
                e4 = e4s.pop(u)
                nslices = (w + 127) // 128
                if qi == 0:
                    accs[ci] = [
                        (apsum.tile([128, 512], f32, tag="acc_a", name="acc_a"),
                         bpsum.tile([128, 512], f32, tag="acc_b", name="acc_b"))
                        for _ in range(nslices)
                    ]
                first = qi == 0
                last = qi == len(QUADS) - 1
                # DoubleRow pairs (and the odd single tile 50 at quad end)
                steps = []
                if qn == 4:
                    steps = [(0, True), (2, True)]
                else:
                    steps = [(0, True), (2, False)]
                for si, (j, dr) in enumerate(steps):
                    nt = q0 + j
                    st = first and si == 0
                    sp = last and si == len(steps) - 1
                    nsz = 128 if dr else NLAST
                    for sl in range(nslices):
                        pw = min(128, w - 128 * sl)
                        acc_a, acc_b = accs[ci][sl]
                        if dr:
                            el = e4[:nsz, j:j + 2, sl * 128:sl * 128 + pw]
                            nc.tensor.matmul(
                                acc_a[:pw, 0:ASPL],
                                lhsT=el,
                                rhs=mvt_sb[:nsz, nt:nt + 2, 0:ASPL],
                                start=st, stop=sp,
                                perf_mode=DR,
                            )
                            nc.tensor.matmul(
                                acc_b[:pw, 0:CVA - ASPL],
                                lhsT=el,
                                rhs=mvt_sb[:nsz, nt:nt + 2, ASPL:CVA],
                                start=st, stop=sp,
                                perf_mode=DR,
                            )
                        else:
                            el = e4[:nsz, j, sl * 128:sl * 128 + pw]
                            nc.tensor.matmul(
                                acc_a[:pw, 0:ASPL],
                                lhsT=el,
                                rhs=mvt_sb[:nsz, nt, 0:ASPL],
                                start=st, stop=sp,
                            )
                            nc.tensor.matmul(
                                acc_b[:pw, 0:CVA - ASPL],
                                lhsT=el,
                                rhs=mvt_sb[:nsz, nt, ASPL:CVA],
                                start=st, stop=sp,
                            )
                if last:
                    for sl in range(nslices):
                        pw = min(128, w - 128 * sl)
                        acc_a, acc_b = accs[ci][sl]
                        o_sb = opool.tile([128, CVA], f32, tag="o", name="o")
                        nc.vector.tensor_copy(out=o_sb[:pw, 0:ASPL],
                                              in_=acc_a[:pw, 0:ASPL])
                        nc.vector.tensor_copy(out=o_sb[:pw, ASPL:CVA],
                                              in_=acc_b[:pw, 0:CVA - ASPL])
                        p0 = ps + sl * 128
                        nc.sync.dma_start(out=mem_d[p0:p0 + pw, :],
                                          in_=o_sb[:pw, :])
                    del accs[ci]

            issue_mm1(0)
            for u in range(len(units)):
                if u + 1 < len(units):
                    issue_mm1(u + 1)
                issue_act(u)
                issue_mm2(u)

    _strip_same_engine_waits(nc)
    if DEDUPE_LDW:
        _dedupe_ldweights(nc)
    nc.compile()
    return nc


def _ldw_key(inst):
    ap = inst.ins[0]
    return repr(ap)


def _dedupe_ldweights(nc):
    """Drop an InstLdweights whose weights AP is identical to the
    immediately-preceding one (only InstMatmult in between): the a/b column
    halves of mm2 share one stationary operand, and a duplicate 256-col
    DoubleRow weight load would make the weight port the bottleneck. The
    dropped load's waits move to the surviving one (deduplicated)."""
    for fn in nc.m.functions:
        for blk in fn.blocks:
            keep = []
            last_ldw = None
            removed_any = False
            for inst in blk.instructions:
                if isinstance(inst, mybir.InstLdweights):
                    if (last_ldw is not None
                            and _ldw_key(inst) == _ldw_key(last_ldw[0])
                            and inst.perf_mode == last_ldw[0].perf_mode):
                        # merge waits into the kept LDW
                        si = getattr(inst, "sync_info", None)
                        if si is not None and si.on_wait:
                            ksi = last_ldw[0].sync_info
                            if ksi is None:
                                last_ldw[0].sync_info = si
                            else:
                                have = {repr(w) for w in ksi.on_wait}
                                for w_ in si.on_wait:
                                    if repr(w_) not in have:
                                        ksi.on_wait.append(w_)
                            assert not (si.on_update or []), (
                                "dropped LDW had sem updates")
                        removed_any = True
                        continue
                    last_ldw = (inst,)
                    keep.append(inst)
                    continue
                if not isinstance(inst, mybir.InstMatmult):
                    last_ldw = None
                keep.append(inst)
            if removed_any:
                blk.instructions[:] = keep


def _strip_same_engine_waits(nc):
    """Drop redundant same-engine semaphore waits on ACT/PE compute
    instructions (each engine executes its queue in order, and TRN2 allows
    only one wait per instruction before EventSemaphore splitting)."""
    prefixes = {
        "EngineType.Activation": "Activation_",
        "EngineType.PE": "PE_",
    }
    kinds = (mybir.InstActivation, mybir.InstMatmult, mybir.InstLdweights)
    for fn in nc.m.functions:
        for blk in fn.blocks:
            for inst in blk.instructions:
                si = getattr(inst, "sync_info", None)
                if si is None or not si.on_wait or not isinstance(inst, kinds):
                    continue
                pref = prefixes.get(str(getattr(inst, "engine", None)))
                if pref is None:
                    continue
                kept = [w for w in si.on_wait
                        if not str(getattr(w, "ant_name", "")).startswith(pref)]
                if len(kept) != len(si.on_wait):
                    si.on_wait = kept


def _get_program():
    if "nc" not in _CACHE:
        _CACHE["nc"] = _build_program()
    return _CACHE["nc"]


def _make_in_maps(mk, mv, qk):
    f8 = ml_dtypes.float8_e4m3
    bf = ml_dtypes.bfloat16
    mkf = np.ascontiguousarray(mk.reshape(B, CK, N))
    mvf = np.ascontiguousarray(mv.reshape(B, CV, N))
    qkf = np.ascontiguousarray(qk.reshape(B, CK, P))
    in_maps = []
    for core in range(8):
        b, half = core // 2, core % 2
        n0, n1 = half * NHALF, (half + 1) * NHALF
        mk_c = mkf[b, :, n0:n1].astype(bf)             # [64, 6480]
        mk_t = np.zeros((128, NT, 128), dtype=bf)
        mk_t[:CK].reshape(CK, NT * 128)[:, :NHALF] = mk_c
        qk_c = np.zeros((128, P), dtype=bf)
        qk_c[:CK] = qkf[b].astype(bf)
        # mvT with the ones column at 512; zeros elsewhere (incl. pad rows
        # and pad tile NT..NTP so the DoubleRow partner contributes nothing)
        mvt = np.zeros((NTP * 128, MVW), dtype=f8)
        mvt[:NHALF, :CV] = mvf[b, :, n0:n1].T.astype(f8)
        mvt[:NHALF, CV] = 1.0
        mvt_c = np.ascontiguousarray(
            mvt.reshape(NTP, 128, MVW).transpose(1, 0, 2))
        in_maps.append({"mk": np.ascontiguousarray(mk_t),
                        "qk": np.ascontiguousarray(qk_c),
                        "mvT": mvt_c})
    return in_maps


def _run(mk, mv, qk, qv, trace=False, **spmd_kwargs):
    nc = _get_program()
    in_maps = _make_in_maps(mk, mv, qk)
    res = run_bass_kernel_spmd(nc, in_maps, list(range(8)), trace=trace,
                               **spmd_kwargs)
    out = np.empty((B, 2 * CV, P), dtype=np.float32)
    for b in range(B):
        m0 = res.results[2 * b]["memT"]
        m1 = res.results[2 * b + 1]["memT"]
        ms = m0 + m1
        out[b, :CV] = (ms[:, :CV] / ms[:, CV][:, None]).T
        out[b, CV:] = qv[b].reshape(CV, P)
    return out.reshape(B, 2 * CV, H, W), res


def kernel(mk, mv, qk, qv):
    out, _ = _run(np.asarray(mk), np.asarray(mv), np.asarray(qk),
                  np.asarray(qv))
    return out
